# revision 10
# baseline (speedup 1.0000x reference)
"""Distributed Trainium2 kernel for the ADD rotation loss.

Math: the reference computes mean_{b,n} || point[b,n] @ (R_pred[b] - R_gt[b]) ||
with R_pred/R_gt rotation matrices. Because both are rotations,

    || p @ (Rp - Rg) || = 2 * | p x qv |,

where qv is the vector part of the relative quaternion q_pred * conj(q_gt).
The pred-side euler angles enter only through cos/sin, which reduce to pure
arithmetic (no arcsin/arctan2 needed); only the gt side needs real sin().
Further, with {E1, E2} an orthogonal basis of the plane perpendicular to qv,
scaled to length |qv| (Duff's branch-free construction),

    | p x qv |^2 = (p . E1)^2 + (p . E2)^2,

which needs only TWO per-point projections (and two squares) instead of the
three cross-product components.

Per core (data-parallel over batch): cast-DMA the point shard f32->bf16 once
from HBM, project onto E1/E2 via TensorEngine matmuls with *diagonal* bf16
stationary matrices (per-batch-row coefficients on the diagonal), square on
ACT, add on DVE, sqrt + per-row accumulate on ACT, and emit per-row partial
sums. The final tiny reduction (8 cores x 128 x 8 values) happens on host.
"""

import sys

for _p in ("/opt/trn_rl_repo", "/root/.axon_site/_ro/trn_rl_repo"):
    if _p not in sys.path:
        sys.path.append(_p)

import math

import numpy as np

import concourse.bacc as bacc
import concourse.tile as tile
from concourse import mybir
from concourse.bass_utils import run_bass_kernel_spmd

NCORES = 8
B = 8192
N = 1024
BSH = B // NCORES          # batch rows per core
G = BSH // 128             # b-groups of 128 rows per core
ROW = 3 * N                # floats per point row
HALF = 3 * (N // 2)        # elements per half row
F32 = mybir.dt.float32
BF16 = mybir.dt.bfloat16
OP = mybir.AluOpType
AF = mybir.ActivationFunctionType

_CACHE = {}


def build_bass():
    nc = bacc.Bacc("TRN2", target_bir_lowering=False, debug=False,
                   num_devices=NCORES)
    pred = nc.declare_dram_parameter("pred", [BSH, 4], F32, isOutput=False)
    mode = nc.declare_dram_parameter("mode", [BSH, 1], F32, isOutput=False)
    gt = nc.declare_dram_parameter("gt", [BSH, 3], F32, isOutput=False)
    point = nc.declare_dram_parameter("point", [BSH, ROW], F32, isOutput=False)
    out = nc.declare_dram_parameter("out", [128, G], F32, isOutput=True)

    with tile.TileContext(nc) as tc:
        with (
            tc.tile_pool(name="coef", bufs=1) as cp,
            tc.tile_pool(name="data", bufs=3) as dp,
            tc.tile_pool(name="diag", bufs=2) as gp,
            tc.tile_pool(name="sq", bufs=2) as qp,
            tc.tile_pool(name="psum", bufs=2, space="PSUM") as pp,
        ):
            uid = [0]

            def ctile(shape, dtype=F32):
                uid[0] += 1
                return cp.tile(shape, dtype, name=f"c{uid[0]}",
                               tag=f"c{uid[0]}")

            def tt(in0, in1, op, shape=None, out=None):
                """out = in0 op in1 (DVE); returns the written AP."""
                if out is None:
                    out = ctile(shape if shape is not None else [128, G])
                nc.vector.tensor_tensor(out=out, in0=in0, in1=in1, op=op)
                return out

            def ts(in0, s1, s2, op0, op1=None, out=None, shape=None):
                if out is None:
                    out = ctile(shape if shape is not None else [128, G])
                if op1 is None:
                    nc.vector.tensor_scalar(out=out, in0=in0, scalar1=s1,
                                            scalar2=None, op0=op0)
                else:
                    nc.vector.tensor_scalar(out=out, in0=in0, scalar1=s1,
                                            scalar2=s2, op0=op0, op1=op1)
                return out

            def recip(in_, shape=None):
                o = ctile(shape if shape is not None else [128, G])
                nc.vector.reciprocal(out=o, in_=in_)
                return o

            _consts = {}

            def constcol(val):
                if val not in _consts:
                    uid[0] += 1
                    t = cp.tile([128, 1], F32, name=f"k{uid[0]}",
                                tag=f"k{uid[0]}")
                    nc.vector.memset(t[:, :], val)
                    _consts[val] = t
                return _consts[val]

            def act(in_, func, scale=1.0, bias=0.0, out=None, shape=None):
                if out is None:
                    out = ctile(shape if shape is not None else [128, G])
                if isinstance(bias, float) and bias != 0.0:
                    bias = constcol(bias)[:, :]
                nc.scalar.activation(out=out, in_=in_, func=func,
                                     scale=scale, bias=bias)
                return out

            # ---- coefficient inputs, transposed so row b=128g+p is at [p,g] ----
            cgt = ctile([128, G, 3])
            nc.sync.dma_start(out=cgt[:, :, :],
                              in_=gt[:, :].rearrange("(g p) c -> p g c", p=128))
            cpred = ctile([128, G, 4])
            nc.sync.dma_start(out=cpred[:, :, :],
                              in_=pred[:, :].rearrange("(g p) c -> p g c", p=128))
            cmode = ctile([128, G, 1])
            nc.sync.dma_start(out=cmode[:, :, :],
                              in_=mode[:, :].rearrange("(g p) c -> p g c", p=128))

            # gt half-angle cos/sin first: the two Sin ops run before anything
            # needing the sqrt table set, so ACT loads each table set once.
            chg = act(cgt[:, :, :], AF.Sin, scale=0.5, bias=math.pi / 2,
                      shape=[128, G, 3])
            shg = act(cgt[:, :, :], AF.Sin, scale=0.5, shape=[128, G, 3])

            # ---- pred side: cos/sin of euler angles, arithmetic only ----
            m1, m2 = cpred[:, :, 0], cpred[:, :, 1]
            m3, m4 = cpred[:, :, 2], cpred[:, :, 3]
            msq = tt(cpred[:, :, :], cpred[:, :, :], OP.mult, shape=[128, G, 4])
            m1sq, m2sq = msq[:, :, 0], msq[:, :, 1]
            m3sq, m4sq = msq[:, :, 2], msq[:, :, 3]
            rxy = tt(m1sq, m2sq, OP.add)
            r = tt(rxy, m3sq, OP.add)
            rinv = recip(r)

            cc = ctile([128, G, 3])   # cos(e1..e3)
            ss = ctile([128, G, 3])   # sin(e1..e3)

            # e2: sin = sgn*sqrt(m3^2/r), cos = sqrt((m1^2+m2^2)/r)
            s2sq = tt(m3sq, rinv, OP.mult)
            c2sq = tt(rxy, rinv, OP.mult)
            s2a = act(s2sq, AF.Sqrt)
            act(c2sq, AF.Sqrt, out=cc[:, :, 1])
            sgn = act(cmode[:, :, 0], AF.Sign, bias=-0.5)
            tt(s2a, sgn, OP.mult, out=ss[:, :, 1])

            # e3: w = m3/(sin(e2)+1e-9); cos/sin = (w, m4)/hyp(w, m4)
            s2e = ts(ss[:, :, 1], 1e-9, None, OP.add)
            s2ei = recip(s2e)
            w = tt(m3, s2ei, OP.mult)
            wsq = tt(w, w, OP.mult)
            h3sq = tt(wsq, m4sq, OP.add)
            h3si = recip(h3sq)
            h3i = act(h3si, AF.Sqrt)
            tt(w, h3i, OP.mult, out=cc[:, :, 2])
            tt(m4, h3i, OP.mult, out=ss[:, :, 2])

            # e1: cos/sin = sign(cos2*cos3) * (m1, m2)/hyp(m1, m2)
            tmp = tt(cc[:, :, 1], cc[:, :, 2], OP.mult)
            sgnt = act(tmp, AF.Sign)
            rxyi = recip(rxy)
            hyi = act(rxyi, AF.Sqrt)
            c1a = tt(m1, hyi, OP.mult)
            s1a = tt(m2, hyi, OP.mult)
            tt(c1a, sgnt, OP.mult, out=cc[:, :, 0])
            tt(s1a, sgnt, OP.mult, out=ss[:, :, 0])

            # clamp cosines into [-1, 1] so the half-angle sqrts stay real
            ts(cc, 1.0, -1.0, OP.min, OP.max, out=cc, shape=None)

            # pred half-angle: ch = sqrt((1+c)/2), sh = sign(s)*sqrt((1-c)/2)
            chp = act(cc, AF.Sqrt, scale=0.5, bias=0.5, shape=[128, G, 3])
            shab = act(cc, AF.Sqrt, scale=-0.5, bias=0.5, shape=[128, G, 3])
            ssgn = act(ss, AF.Sign, shape=[128, G, 3])
            shp = tt(shab, ssgn, OP.mult, shape=[128, G, 3])

            # ---- quaternions: q = qx(e1) * qy(e2) * qz(e3) ----
            def quat_xyz(ch, sh):
                c1h, s1h = ch[:, :, 0], sh[:, :, 0]
                c2h, s2h = ch[:, :, 1], sh[:, :, 1]
                c3h, s3h = ch[:, :, 2], sh[:, :, 2]
                w12 = tt(c1h, c2h, OP.mult)
                x12 = tt(s1h, c2h, OP.mult)
                y12 = tt(c1h, s2h, OP.mult)
                z12 = tt(s1h, s2h, OP.mult)
                wq = tt(tt(w12, c3h, OP.mult), tt(z12, s3h, OP.mult), OP.subtract)
                xq = tt(tt(x12, c3h, OP.mult), tt(y12, s3h, OP.mult), OP.add)
                yq = tt(tt(y12, c3h, OP.mult), tt(x12, s3h, OP.mult), OP.subtract)
                zq = tt(tt(w12, s3h, OP.mult), tt(z12, c3h, OP.mult), OP.add)
                return wq, xq, yq, zq

            wp, xp, yp, zp = quat_xyz(chp, shp)
            wg, xg, yg, zg = quat_xyz(chg, shg)

            # qv = vec(q_pred * conj(q_gt))
            def sub4(a, b, c, d, out):
                # out = a - b - (c - d)
                tt(tt(a, b, OP.subtract), tt(c, d, OP.subtract),
                   OP.subtract, out=out)

            qv = ctile([128, G, 3])
            sub4(tt(xp, wg, OP.mult), tt(wp, xg, OP.mult),
                 tt(yp, zg, OP.mult), tt(zp, yg, OP.mult), qv[:, :, 0])
            sub4(tt(yp, wg, OP.mult), tt(wp, yg, OP.mult),
                 tt(zp, xg, OP.mult), tt(xp, zg, OP.mult), qv[:, :, 1])
            sub4(tt(zp, wg, OP.mult), tt(wp, zg, OP.mult),
                 tt(xp, yg, OP.mult), tt(yp, xg, OP.mult), qv[:, :, 2])

            # ---- orthogonal basis {E1, E2} perp to qv, |E| = |qv| (Duff) ----
            qq = tt(qv[:, :, :], qv[:, :, :], OP.mult, shape=[128, G, 3])
            q2 = tt(tt(qq[:, :, 0], qq[:, :, 1], OP.add), qq[:, :, 2], OP.add)
            q2e = ts(q2, 1e-30, None, OP.add)
            q2ei = recip(q2e)
            rs = act(q2ei, AF.Sqrt)            # 1/|qv|
            qn = tt(q2e, rs, OP.mult)          # |qv|
            ux = tt(qv[:, :, 0], rs, OP.mult)
            uy = tt(qv[:, :, 1], rs, OP.mult)
            uz = tt(qv[:, :, 2], rs, OP.mult)
            s = act(uz, AF.Sign)
            spz = tt(s, uz, OP.add)
            a = ts(recip(spz), -1.0, None, OP.mult)   # a = -1/(s+uz)
            bb = tt(tt(ux, uy, OP.mult), a, OP.mult)
            e1 = ctile([128, G, 3])
            e2 = ctile([128, G, 3])
            # e1 = (1 + s*ux^2*a, s*b, -s*ux); e2 = (b, s + uy^2*a, -uy)
            ux2a = tt(tt(ux, ux, OP.mult), a, OP.mult)
            ts(tt(ux2a, s, OP.mult), 1.0, None, OP.add, out=e1[:, :, 0])
            tt(s, bb, OP.mult, out=e1[:, :, 1])
            ts(tt(s, ux, OP.mult), -1.0, None, OP.mult, out=e1[:, :, 2])
            uy2a = tt(tt(uy, uy, OP.mult), a, OP.mult)
            tt(uy2a, s, OP.add, out=e2[:, :, 1])
            ts(uy, -1.0, None, OP.mult, out=e2[:, :, 2])
            # e2x = b (scaled next); copy via scale stage
            E1 = ctile([128, G, 3])
            E2 = ctile([128, G, 3])
            for c in range(3):
                tt(e1[:, :, c], qn, OP.mult, out=E1[:, :, c])
            tt(bb, qn, OP.mult, out=E2[:, :, 0])
            tt(e2[:, :, 1], qn, OP.mult, out=E2[:, :, 1])
            tt(e2[:, :, 2], qn, OP.mult, out=E2[:, :, 2])

            # ---- identity matrix for diag stationaries ----
            ones = cp.tile([128, 128], F32, name="ones", tag="ones")
            nc.vector.memset(ones[:, :], 1.0)
            ident = cp.tile([128, 128], F32, name="ident", tag="ident")
            nc.gpsimd.affine_select(out=ident[:], in_=ones[:],
                                    pattern=[[-1, 128]],
                                    compare_op=OP.is_equal, fill=0.0,
                                    base=0, channel_multiplier=1)

            acc = cp.tile([128, G], F32, name="acc", tag="acc")

            # ---- main loop over b-groups ----
            for g in range(G):
                T = dp.tile([128, ROW], BF16, name="T", tag="T")
                nc.gpsimd.dma_start(out=T[:, :],
                                    in_=point[g * 128:(g + 1) * 128, :])

                diags = []
                for nm, E in (("d1", E1), ("d2", E2)):
                    row = []
                    for c in range(3):
                        d = gp.tile([128, 128], BF16, name=f"{nm}{c}",
                                    tag=f"{nm}{c}")
                        nc.vector.tensor_scalar(out=d[:], in0=ident[:],
                                                scalar1=E[:, g:g + 1, c],
                                                scalar2=None, op0=OP.mult)
                        row.append(d)
                    diags.append(row)

                pv1 = pp.tile([128, N], F32, name="pv1", tag="pv1")
                pv2 = pp.tile([128, N], F32, name="pv2", tag="pv2")
                for h in range(2):
                    o = h * HALF
                    col = slice(h * (N // 2), (h + 1) * (N // 2))
                    views = [T[:, o + c:o + HALF:3] for c in range(3)]
                    for ps, drow in ((pv1, diags[0]), (pv2, diags[1])):
                        nc.tensor.matmul(out=ps[:, col], lhsT=drow[0][:],
                                         rhs=views[0], start=True, stop=False)
                        nc.tensor.matmul(out=ps[:, col], lhsT=drow[1][:],
                                         rhs=views[1], start=False, stop=False)
                        nc.tensor.matmul(out=ps[:, col], lhsT=drow[2][:],
                                         rhs=views[2], start=False, stop=True)

                sq1 = qp.tile([128, N], BF16, name="sq1", tag="sq1")
                sq2 = qp.tile([128, N], BF16, name="sq2", tag="sq2")
                nc.scalar.activation(out=sq1[:], in_=pv1[:], func=AF.Square)
                nc.scalar.activation(out=sq2[:], in_=pv2[:], func=AF.Square)

                stot = qp.tile([128, N], BF16, name="stot", tag="stot")
                nc.vector.tensor_tensor(out=stot[:], in0=sq1[:], in1=sq2[:],
                                        op=OP.add)

                dists = qp.tile([128, N], BF16, name="dists", tag="dists")
                nc.scalar.activation(out=dists[:], in_=stot[:], func=AF.Sqrt,
                                     scale=4.0, accum_out=acc[:, g:g + 1])

            nc.sync.dma_start(out=out[:, :], in_=acc[:, :])

    nc.finalize()
    return nc


def _get_nc():
    if "nc" not in _CACHE:
        _CACHE["nc"] = build_bass()
    return _CACHE["nc"]


def kernel(pred, mode, gt, point, **run_kwargs):
    nc = _get_nc()
    in_maps = []
    for c in range(NCORES):
        sl = slice(c * BSH, (c + 1) * BSH)
        in_maps.append({
            "pred": np.ascontiguousarray(pred[sl], dtype=np.float32),
            "mode": np.ascontiguousarray(mode[sl], dtype=np.float32).reshape(BSH, 1),
            "gt": np.ascontiguousarray(gt[sl], dtype=np.float32),
            "point": np.ascontiguousarray(point[sl], dtype=np.float32).reshape(BSH, ROW),
        })
    res = run_bass_kernel_spmd(nc, in_maps, core_ids=list(range(NCORES)),
                               **run_kwargs)
    total = sum(float(r["out"].astype(np.float64).sum()) for r in res.results)
    result = np.float32(total / (B * N))
    if run_kwargs:
        return result, res
    return result


# revision 11
# speedup vs baseline: 1.0535x; 1.0535x over previous
"""Distributed Trainium2 kernel for the ADD rotation loss.

Math: the reference computes mean_{b,n} || point[b,n] @ (R_pred[b] - R_gt[b]) ||
with R_pred/R_gt rotation matrices. Because both are rotations,

    || p @ (Rp - Rg) || = 2 * | p x qv |,

where qv is the vector part of the relative quaternion q_pred * conj(q_gt).
The pred-side euler angles enter only through cos/sin, which reduce to pure
arithmetic (no arcsin/arctan2 needed); only the gt side needs real sin().
Further, with {E1, E2} an orthogonal basis of the plane perpendicular to qv,
scaled to length |qv| (Duff's branch-free construction),

    | p x qv |^2 = (p . E1)^2 + (p . E2)^2,

which needs only TWO per-point projections (and two squares) instead of the
three cross-product components.

Per core (data-parallel over batch): cast-DMA the point shard f32->bf16 once
from HBM, project onto E1/E2 via TensorEngine matmuls with *diagonal* bf16
stationary matrices (per-batch-row coefficients on the diagonal), square on
ACT, add on DVE, sqrt + per-row accumulate on ACT, and emit per-row partial
sums. The final tiny reduction (8 cores x 128 x 8 values) happens on host.
"""

import sys

for _p in ("/opt/trn_rl_repo", "/root/.axon_site/_ro/trn_rl_repo"):
    if _p not in sys.path:
        sys.path.append(_p)

import math

import numpy as np

import concourse.bacc as bacc
import concourse.tile as tile
from concourse import mybir
from concourse.bass_utils import run_bass_kernel_spmd

NCORES = 8
B = 8192
N = 1024
BSH = B // NCORES          # batch rows per core
G = BSH // 128             # b-groups of 128 rows per core
ROW = 3 * N                # floats per point row
HALF = 3 * (N // 2)        # elements per half row
F32 = mybir.dt.float32
BF16 = mybir.dt.bfloat16
OP = mybir.AluOpType
AF = mybir.ActivationFunctionType

_CACHE = {}


def build_bass():
    nc = bacc.Bacc("TRN2", target_bir_lowering=False, debug=False,
                   num_devices=NCORES)
    pred = nc.declare_dram_parameter("pred", [BSH, 4], F32, isOutput=False)
    mode = nc.declare_dram_parameter("mode", [BSH, 1], F32, isOutput=False)
    gt = nc.declare_dram_parameter("gt", [BSH, 3], F32, isOutput=False)
    point = nc.declare_dram_parameter("point", [BSH, ROW], F32, isOutput=False)
    out = nc.declare_dram_parameter("out", [128, G], F32, isOutput=True)

    with tile.TileContext(nc) as tc:
        with (
            tc.tile_pool(name="coef", bufs=1) as cp,
            tc.tile_pool(name="data", bufs=3) as dp,
            tc.tile_pool(name="diag", bufs=2) as gp,
            tc.tile_pool(name="sq", bufs=2) as qp,
            tc.tile_pool(name="psum", bufs=2, space="PSUM") as pp,
        ):
            uid = [0]

            def ctile(shape, dtype=F32):
                uid[0] += 1
                return cp.tile(shape, dtype, name=f"c{uid[0]}",
                               tag=f"c{uid[0]}")

            def tt(in0, in1, op, shape=None, out=None):
                """out = in0 op in1 (DVE); returns the written AP."""
                if out is None:
                    out = ctile(shape if shape is not None else [128, G])
                nc.vector.tensor_tensor(out=out, in0=in0, in1=in1, op=op)
                return out

            def ts(in0, s1, s2, op0, op1=None, out=None, shape=None):
                if out is None:
                    out = ctile(shape if shape is not None else [128, G])
                if op1 is None:
                    nc.vector.tensor_scalar(out=out, in0=in0, scalar1=s1,
                                            scalar2=None, op0=op0)
                else:
                    nc.vector.tensor_scalar(out=out, in0=in0, scalar1=s1,
                                            scalar2=s2, op0=op0, op1=op1)
                return out

            def recip(in_, shape=None):
                o = ctile(shape if shape is not None else [128, G])
                nc.vector.reciprocal(out=o, in_=in_)
                return o

            _consts = {}

            def constcol(val):
                if val not in _consts:
                    uid[0] += 1
                    t = cp.tile([128, 1], F32, name=f"k{uid[0]}",
                                tag=f"k{uid[0]}")
                    nc.vector.memset(t[:, :], val)
                    _consts[val] = t
                return _consts[val]

            def act(in_, func, scale=1.0, bias=0.0, out=None, shape=None):
                if out is None:
                    out = ctile(shape if shape is not None else [128, G])
                if isinstance(bias, float) and bias != 0.0:
                    bias = constcol(bias)[:, :]
                nc.scalar.activation(out=out, in_=in_, func=func,
                                     scale=scale, bias=bias)
                return out

            # ---- coefficient inputs, transposed so row b=128g+p is at [p,g] ----
            # Row assignment: batch row b = 8p + g lives on partition p,
            # group g. This makes every coefficient load a small fully
            # contiguous DMA (the whole DRAM tensor viewed as [128, G*c]).
            cgt = ctile([128, G, 3])
            nc.sync.dma_start(out=cgt[:, :, :],
                              in_=gt[:, :].rearrange("(p g) c -> p g c", p=128))
            cpred = ctile([128, G, 4])
            nc.sync.dma_start(out=cpred[:, :, :],
                              in_=pred[:, :].rearrange("(p g) c -> p g c", p=128))
            cmode = ctile([128, G, 1])
            nc.sync.dma_start(out=cmode[:, :, :],
                              in_=mode[:, :].rearrange("(p g) c -> p g c", p=128))

            # gt half-angle cos/sin first: the two Sin ops run before anything
            # needing the sqrt table set, so ACT loads each table set once.
            chg = act(cgt[:, :, :], AF.Sin, scale=0.5, bias=math.pi / 2,
                      shape=[128, G, 3])
            shg = act(cgt[:, :, :], AF.Sin, scale=0.5, shape=[128, G, 3])

            # ---- pred side: cos/sin of euler angles, arithmetic only ----
            m1, m2 = cpred[:, :, 0], cpred[:, :, 1]
            m3, m4 = cpred[:, :, 2], cpred[:, :, 3]
            msq = tt(cpred[:, :, :], cpred[:, :, :], OP.mult, shape=[128, G, 4])
            m1sq, m2sq = msq[:, :, 0], msq[:, :, 1]
            m3sq, m4sq = msq[:, :, 2], msq[:, :, 3]
            rxy = tt(m1sq, m2sq, OP.add)
            r = tt(rxy, m3sq, OP.add)
            rinv = recip(r)

            cc = ctile([128, G, 3])   # cos(e1..e3)
            ss = ctile([128, G, 3])   # sin(e1..e3)

            # e2: sin = sgn*sqrt(m3^2/r), cos = sqrt((m1^2+m2^2)/r)
            s2sq = tt(m3sq, rinv, OP.mult)
            c2sq = tt(rxy, rinv, OP.mult)
            s2a = act(s2sq, AF.Sqrt)
            act(c2sq, AF.Sqrt, out=cc[:, :, 1])
            sgn = act(cmode[:, :, 0], AF.Sign, bias=-0.5)
            tt(s2a, sgn, OP.mult, out=ss[:, :, 1])

            # e3: w = m3/(sin(e2)+1e-9); cos/sin = (w, m4)/hyp(w, m4)
            s2e = ts(ss[:, :, 1], 1e-9, None, OP.add)
            s2ei = recip(s2e)
            w = tt(m3, s2ei, OP.mult)
            wsq = tt(w, w, OP.mult)
            h3sq = tt(wsq, m4sq, OP.add)
            h3si = recip(h3sq)
            h3i = act(h3si, AF.Sqrt)
            tt(w, h3i, OP.mult, out=cc[:, :, 2])
            tt(m4, h3i, OP.mult, out=ss[:, :, 2])

            # e1: cos/sin = sign(cos2*cos3) * (m1, m2)/hyp(m1, m2)
            tmp = tt(cc[:, :, 1], cc[:, :, 2], OP.mult)
            sgnt = act(tmp, AF.Sign)
            rxyi = recip(rxy)
            hyi = act(rxyi, AF.Sqrt)
            c1a = tt(m1, hyi, OP.mult)
            s1a = tt(m2, hyi, OP.mult)
            tt(c1a, sgnt, OP.mult, out=cc[:, :, 0])
            tt(s1a, sgnt, OP.mult, out=ss[:, :, 0])

            # clamp cosines into [-1, 1] so the half-angle sqrts stay real
            ts(cc, 1.0, -1.0, OP.min, OP.max, out=cc, shape=None)

            # pred half-angle: ch = sqrt((1+c)/2), sh = sign(s)*sqrt((1-c)/2)
            chp = act(cc, AF.Sqrt, scale=0.5, bias=0.5, shape=[128, G, 3])
            shab = act(cc, AF.Sqrt, scale=-0.5, bias=0.5, shape=[128, G, 3])
            ssgn = act(ss, AF.Sign, shape=[128, G, 3])
            shp = tt(shab, ssgn, OP.mult, shape=[128, G, 3])

            # ---- quaternions: q = qx(e1) * qy(e2) * qz(e3) ----
            def quat_xyz(ch, sh):
                c1h, s1h = ch[:, :, 0], sh[:, :, 0]
                c2h, s2h = ch[:, :, 1], sh[:, :, 1]
                c3h, s3h = ch[:, :, 2], sh[:, :, 2]
                w12 = tt(c1h, c2h, OP.mult)
                x12 = tt(s1h, c2h, OP.mult)
                y12 = tt(c1h, s2h, OP.mult)
                z12 = tt(s1h, s2h, OP.mult)
                wq = tt(tt(w12, c3h, OP.mult), tt(z12, s3h, OP.mult), OP.subtract)
                xq = tt(tt(x12, c3h, OP.mult), tt(y12, s3h, OP.mult), OP.add)
                yq = tt(tt(y12, c3h, OP.mult), tt(x12, s3h, OP.mult), OP.subtract)
                zq = tt(tt(w12, s3h, OP.mult), tt(z12, c3h, OP.mult), OP.add)
                return wq, xq, yq, zq

            wp, xp, yp, zp = quat_xyz(chp, shp)
            wg, xg, yg, zg = quat_xyz(chg, shg)

            # qv = vec(q_pred * conj(q_gt))
            def sub4(a, b, c, d, out):
                # out = a - b - (c - d)
                tt(tt(a, b, OP.subtract), tt(c, d, OP.subtract),
                   OP.subtract, out=out)

            qv = ctile([128, G, 3])
            sub4(tt(xp, wg, OP.mult), tt(wp, xg, OP.mult),
                 tt(yp, zg, OP.mult), tt(zp, yg, OP.mult), qv[:, :, 0])
            sub4(tt(yp, wg, OP.mult), tt(wp, yg, OP.mult),
                 tt(zp, xg, OP.mult), tt(xp, zg, OP.mult), qv[:, :, 1])
            sub4(tt(zp, wg, OP.mult), tt(wp, zg, OP.mult),
                 tt(xp, yg, OP.mult), tt(yp, xg, OP.mult), qv[:, :, 2])

            # ---- orthogonal basis {E1, E2} perp to qv, |E| = |qv| (Duff) ----
            qq = tt(qv[:, :, :], qv[:, :, :], OP.mult, shape=[128, G, 3])
            q2 = tt(tt(qq[:, :, 0], qq[:, :, 1], OP.add), qq[:, :, 2], OP.add)
            q2e = ts(q2, 1e-30, None, OP.add)
            q2ei = recip(q2e)
            rs = act(q2ei, AF.Sqrt)            # 1/|qv|
            qn = tt(q2e, rs, OP.mult)          # |qv|
            ux = tt(qv[:, :, 0], rs, OP.mult)
            uy = tt(qv[:, :, 1], rs, OP.mult)
            uz = tt(qv[:, :, 2], rs, OP.mult)
            s = act(uz, AF.Sign)
            spz = tt(s, uz, OP.add)
            a = ts(recip(spz), -1.0, None, OP.mult)   # a = -1/(s+uz)
            bb = tt(tt(ux, uy, OP.mult), a, OP.mult)
            e1 = ctile([128, G, 3])
            e2 = ctile([128, G, 3])
            # e1 = (1 + s*ux^2*a, s*b, -s*ux); e2 = (b, s + uy^2*a, -uy)
            ux2a = tt(tt(ux, ux, OP.mult), a, OP.mult)
            ts(tt(ux2a, s, OP.mult), 1.0, None, OP.add, out=e1[:, :, 0])
            tt(s, bb, OP.mult, out=e1[:, :, 1])
            ts(tt(s, ux, OP.mult), -1.0, None, OP.mult, out=e1[:, :, 2])
            uy2a = tt(tt(uy, uy, OP.mult), a, OP.mult)
            tt(uy2a, s, OP.add, out=e2[:, :, 1])
            ts(uy, -1.0, None, OP.mult, out=e2[:, :, 2])
            # e2x = b (scaled next); copy via scale stage
            E1 = ctile([128, G, 3])
            E2 = ctile([128, G, 3])
            for c in range(3):
                tt(e1[:, :, c], qn, OP.mult, out=E1[:, :, c])
            tt(bb, qn, OP.mult, out=E2[:, :, 0])
            tt(e2[:, :, 1], qn, OP.mult, out=E2[:, :, 1])
            tt(e2[:, :, 2], qn, OP.mult, out=E2[:, :, 2])

            # ---- identity matrix for diag stationaries ----
            ones = cp.tile([128, 128], F32, name="ones", tag="ones")
            nc.vector.memset(ones[:, :], 1.0)
            ident = cp.tile([128, 128], F32, name="ident", tag="ident")
            nc.gpsimd.affine_select(out=ident[:], in_=ones[:],
                                    pattern=[[-1, 128]],
                                    compare_op=OP.is_equal, fill=0.0,
                                    base=0, channel_multiplier=1)

            acc = cp.tile([128, G], F32, name="acc", tag="acc")

            # ---- main loop over b-groups ----
            for g in range(G):
                T = dp.tile([128, ROW], BF16, name="T", tag="T")
                nc.gpsimd.dma_start(out=T[:, :],
                                    in_=point[g:BSH:G, :])

                diags = []
                for nm, E in (("d1", E1), ("d2", E2)):
                    row = []
                    for c in range(3):
                        d = gp.tile([128, 128], BF16, name=f"{nm}{c}",
                                    tag=f"{nm}{c}")
                        nc.vector.tensor_scalar(out=d[:], in0=ident[:],
                                                scalar1=E[:, g:g + 1, c],
                                                scalar2=None, op0=OP.mult)
                        row.append(d)
                    diags.append(row)

                pv1 = pp.tile([128, N], F32, name="pv1", tag="pv1")
                pv2 = pp.tile([128, N], F32, name="pv2", tag="pv2")
                for h in range(2):
                    o = h * HALF
                    col = slice(h * (N // 2), (h + 1) * (N // 2))
                    views = [T[:, o + c:o + HALF:3] for c in range(3)]
                    for ps, drow in ((pv1, diags[0]), (pv2, diags[1])):
                        # The stationary is diagonal, so split into 4
                        # concurrent 32x32 blocks on distinct row/col
                        # quadrants of the PE array (tile_position).
                        for c in range(3):
                            for q in range(4):
                                qs = slice(32 * q, 32 * (q + 1))
                                nc.tensor.matmul(
                                    out=ps[qs, col],
                                    lhsT=drow[c][qs, qs],
                                    rhs=views[c][qs, :],
                                    start=(c == 0), stop=(c == 2),
                                    tile_position=(32 * q, 32 * q))

                sq1 = qp.tile([128, N], BF16, name="sq1", tag="sq1")
                sq2 = qp.tile([128, N], BF16, name="sq2", tag="sq2")
                nc.scalar.activation(out=sq1[:], in_=pv1[:], func=AF.Square)
                nc.scalar.activation(out=sq2[:], in_=pv2[:], func=AF.Square)

                stot = qp.tile([128, N], BF16, name="stot", tag="stot")
                nc.vector.tensor_tensor(out=stot[:], in0=sq1[:], in1=sq2[:],
                                        op=OP.add)

                dists = qp.tile([128, N], BF16, name="dists", tag="dists")
                nc.scalar.activation(out=dists[:], in_=stot[:], func=AF.Sqrt,
                                     scale=4.0, accum_out=acc[:, g:g + 1])

            nc.sync.dma_start(out=out[:, :], in_=acc[:, :])

    nc.finalize()
    return nc


def _get_nc():
    if "nc" not in _CACHE:
        _CACHE["nc"] = build_bass()
    return _CACHE["nc"]


def kernel(pred, mode, gt, point, **run_kwargs):
    nc = _get_nc()
    in_maps = []
    for c in range(NCORES):
        sl = slice(c * BSH, (c + 1) * BSH)
        in_maps.append({
            "pred": np.ascontiguousarray(pred[sl], dtype=np.float32),
            "mode": np.ascontiguousarray(mode[sl], dtype=np.float32).reshape(BSH, 1),
            "gt": np.ascontiguousarray(gt[sl], dtype=np.float32),
            "point": np.ascontiguousarray(point[sl], dtype=np.float32).reshape(BSH, ROW),
        })
    res = run_bass_kernel_spmd(nc, in_maps, core_ids=list(range(NCORES)),
                               **run_kwargs)
    total = sum(float(r["out"].astype(np.float64).sum()) for r in res.results)
    result = np.float32(total / (B * N))
    if run_kwargs:
        return result, res
    return result


# revision 14
# speedup vs baseline: 1.1258x; 1.0686x over previous
"""Distributed Trainium2 kernel for the ADD rotation loss.

Math: the reference computes mean_{b,n} || point[b,n] @ (R_pred[b] - R_gt[b]) ||
with R_pred/R_gt rotation matrices. Because both are rotations,

    || p @ (Rp - Rg) || = 2 * | p x qv |,

where qv is the vector part of the relative quaternion q_pred * conj(q_gt).
The pred-side euler angles enter only through cos/sin, which reduce to pure
arithmetic (no arcsin/arctan2 needed); only the gt side needs real sin().
With {E1, E2} an orthogonal basis of the plane perpendicular to qv scaled to
length |qv| (from the Householder reflection that maps qv to the z axis),

    | p x qv |^2 = (p . E1)^2 + (p . E2)^2,

i.e. TWO per-point projections + squares instead of three cross components.

Per core (data-parallel over batch): cast-DMA the point shard f32->bf16 once
from HBM, project onto E1/E2 via TensorEngine matmuls whose stationary is a
*diagonal* bf16 matrix (per-batch-row coefficient on the diagonal), square on
ACT (one [128,2048] op per group), add halves on DVE, sqrt + per-row
accumulate on ACT, and emit per-row partial sums. The final tiny reduction
(8 cores x 128 x 8 values) happens on the host.
"""

import sys

for _p in ("/opt/trn_rl_repo", "/root/.axon_site/_ro/trn_rl_repo"):
    if _p not in sys.path:
        sys.path.append(_p)

import math

import numpy as np

import concourse.bacc as bacc
import concourse.tile as tile
from concourse import mybir
from concourse.bass_utils import run_bass_kernel_spmd

NCORES = 8
B = 8192
N = 1024
BSH = B // NCORES          # batch rows per core
G = BSH // 128             # b-groups of 128 rows per core
ROW = 3 * N                # elements per point row
HALF = 3 * (N // 2)        # elements per half row
F32 = mybir.dt.float32
BF16 = mybir.dt.bfloat16
OP = mybir.AluOpType
AF = mybir.ActivationFunctionType

_CACHE = {}


def build_bass():
    nc = bacc.Bacc("TRN2", target_bir_lowering=False, debug=False,
                   num_devices=NCORES)
    pred = nc.declare_dram_parameter("pred", [BSH, 4], F32, isOutput=False)
    mode = nc.declare_dram_parameter("mode", [BSH, 1], F32, isOutput=False)
    gt = nc.declare_dram_parameter("gt", [BSH, 3], F32, isOutput=False)
    point = nc.declare_dram_parameter("point", [BSH, ROW], F32, isOutput=False)
    out = nc.declare_dram_parameter("out", [128, G], F32, isOutput=True)

    with tile.TileContext(nc) as tc:
        with (
            tc.tile_pool(name="coef", bufs=1) as cp,
            tc.tile_pool(name="data", bufs=3) as dp,
            tc.tile_pool(name="diag", bufs=2) as gp,
            tc.tile_pool(name="sq", bufs=2) as qp,
            tc.tile_pool(name="psum", bufs=2, space="PSUM") as pp,
        ):
            uid = [0]

            def ctile(shape, dtype=F32):
                uid[0] += 1
                return cp.tile(shape, dtype, name=f"c{uid[0]}",
                               tag=f"c{uid[0]}")

            def tt(in0, in1, op, shape=None, out=None):
                """out = in0 op in1 (DVE); returns the written AP."""
                if out is None:
                    out = ctile(shape if shape is not None else [128, G])
                nc.vector.tensor_tensor(out=out, in0=in0, in1=in1, op=op)
                return out

            def ts(in0, s1, s2, op0, op1=None, out=None, shape=None):
                if out is None:
                    out = ctile(shape if shape is not None else [128, G])
                if op1 is None:
                    nc.vector.tensor_scalar(out=out, in0=in0, scalar1=s1,
                                            scalar2=None, op0=op0)
                else:
                    nc.vector.tensor_scalar(out=out, in0=in0, scalar1=s1,
                                            scalar2=s2, op0=op0, op1=op1)
                return out

            def sign_dve(in_, shape=None):
                # sign(x) in {-1, +1} without touching ACT:
                # ((x is_ge 0) * 2) - 1
                h = ts(in_, 0.0, 2.0, OP.is_ge, OP.mult, shape=shape)
                return ts(h, -1.0, None, OP.add, out=None, shape=shape)

            def recip(in_, shape=None):
                o = ctile(shape if shape is not None else [128, G])
                nc.vector.reciprocal(out=o, in_=in_)
                return o

            _consts = {}

            def constcol(val):
                if val not in _consts:
                    uid[0] += 1
                    t = cp.tile([128, 1], F32, name=f"k{uid[0]}",
                                tag=f"k{uid[0]}")
                    nc.vector.memset(t[:, :], val)
                    _consts[val] = t
                return _consts[val]

            def act(in_, func, scale=1.0, bias=0.0, out=None, shape=None):
                if out is None:
                    out = ctile(shape if shape is not None else [128, G])
                if isinstance(bias, float) and bias != 0.0:
                    bias = constcol(bias)[:, :]
                nc.scalar.activation(out=out, in_=in_, func=func,
                                     scale=scale, bias=bias)
                return out

            # ---- coefficient inputs ----
            # Row assignment: batch row b = G*p + g lives at [partition p,
            # group g], so each load is one small fully-contiguous DMA.
            cgt = ctile([128, G, 3])
            nc.sync.dma_start(out=cgt[:, :, :],
                              in_=gt[:, :].rearrange("(p g) c -> p g c", p=128))
            cpred = ctile([128, G, 4])
            nc.sync.dma_start(out=cpred[:, :, :],
                              in_=pred[:, :].rearrange("(p g) c -> p g c", p=128))
            cmode = ctile([128, G, 1])
            nc.sync.dma_start(out=cmode[:, :, :],
                              in_=mode[:, :].rearrange("(p g) c -> p g c", p=128))

            # gt half-angle cos/sin first: ACT loads the trig table once,
            # then switches to the sqrt set for the rest of the kernel.
            chg = act(cgt[:, :, :], AF.Sin, scale=0.5, bias=math.pi / 2,
                      shape=[128, G, 3])
            shg = act(cgt[:, :, :], AF.Sin, scale=0.5, shape=[128, G, 3])

            # ---- pred side: cos/sin of euler angles, arithmetic only ----
            m1, m2 = cpred[:, :, 0], cpred[:, :, 1]
            m3, m4 = cpred[:, :, 2], cpred[:, :, 3]
            # off-critical-path signs (inputs only)
            sgn = act(cmode[:, :, 0], AF.Sign, bias=-0.5)   # mode>0.5 -> +1
            sm3 = sign_dve(m3)
            ssm = tt(sgn, sm3, OP.mult)        # sgn*sign(m3)
            am3 = tt(m3, sm3, OP.mult)         # |m3|
            ams = tt(am3, sgn, OP.mult)        # sgn*|m3|

            msq = tt(cpred[:, :, :], cpred[:, :, :], OP.mult, shape=[128, G, 4])
            m1sq, m2sq = msq[:, :, 0], msq[:, :, 1]
            m3sq, m4sq = msq[:, :, 2], msq[:, :, 3]
            rxy = tt(m1sq, m2sq, OP.add)
            r = tt(rxy, m3sq, OP.add)

            cc = ctile([128, G, 3])   # cos(e1..e3)
            ss = ctile([128, G, 3])   # sin(e1..e3)

            # e2: sin = sgn*|m3|*rsqrt(r), cos = sqrt(rxy)*rsqrt(r)
            rsr = act(recip(r), AF.Sqrt)       # rsqrt(r)
            rt_xy = act(rxy, AF.Sqrt)          # sqrt(rxy)
            tt(rt_xy, rsr, OP.mult, out=cc[:, :, 1])
            tt(ams, rsr, OP.mult, out=ss[:, :, 1])

            # e3 (w = m3/(sin(e2)+1e-9) ~= sgn*sign(m3)*sqrt(r)):
            # cos/sin = (w, m4)/hyp(w, m4), w^2 = r
            rt = act(r, AF.Sqrt)               # sqrt(r)
            h3sq = tt(r, m4sq, OP.add)
            h3i = act(recip(h3sq), AF.Sqrt)    # rsqrt(r + m4^2)
            wh = tt(rt, h3i, OP.mult)          # sqrt(r)*h3i
            tt(wh, ssm, OP.mult, out=cc[:, :, 2])
            tt(m4, h3i, OP.mult, out=ss[:, :, 2])

            # e1: cos/sin = sign(cos2*cos3) * (m1, m2)/hyp(m1, m2)
            hyi = act(recip(rxy), AF.Sqrt)     # rsqrt(m1^2+m2^2)
            c1a = tt(m1, hyi, OP.mult)
            s1a = tt(m2, hyi, OP.mult)
            tmp = tt(cc[:, :, 1], cc[:, :, 2], OP.mult)
            sgnt = sign_dve(tmp)
            tt(c1a, sgnt, OP.mult, out=cc[:, :, 0])
            tt(s1a, sgnt, OP.mult, out=ss[:, :, 0])

            # clamp cosines into [-1, 1] so the half-angle sqrts stay real
            ts(cc, 1.0, -1.0, OP.min, OP.max, out=cc, shape=None)

            # pred half-angle: ch = sqrt((1+c)/2), sh = sign(s)*sqrt((1-c)/2)
            chp = act(cc, AF.Sqrt, scale=0.5, bias=0.5, shape=[128, G, 3])
            shab = act(cc, AF.Sqrt, scale=-0.5, bias=0.5, shape=[128, G, 3])
            ssg1 = ts(ss, 0.0, 2.0, OP.is_ge, OP.mult, shape=[128, G, 3])
            ssgn = ts(ssg1, -1.0, None, OP.add, shape=[128, G, 3])
            shp = tt(shab, ssgn, OP.mult, shape=[128, G, 3])

            # ---- quaternions: q = qx(e1) * qy(e2) * qz(e3) ----
            def quat_xyz(ch, sh):
                c1h, s1h = ch[:, :, 0], sh[:, :, 0]
                c2h, s2h = ch[:, :, 1], sh[:, :, 1]
                c3h, s3h = ch[:, :, 2], sh[:, :, 2]
                w12 = tt(c1h, c2h, OP.mult)
                x12 = tt(s1h, c2h, OP.mult)
                y12 = tt(c1h, s2h, OP.mult)
                z12 = tt(s1h, s2h, OP.mult)
                wq = tt(tt(w12, c3h, OP.mult), tt(z12, s3h, OP.mult), OP.subtract)
                xq = tt(tt(x12, c3h, OP.mult), tt(y12, s3h, OP.mult), OP.add)
                yq = tt(tt(y12, c3h, OP.mult), tt(x12, s3h, OP.mult), OP.subtract)
                zq = tt(tt(w12, s3h, OP.mult), tt(z12, c3h, OP.mult), OP.add)
                return wq, xq, yq, zq

            wp, xp, yp, zp = quat_xyz(chp, shp)
            wg, xg, yg, zg = quat_xyz(chg, shg)

            # qv = vec(q_pred * conj(q_gt))
            def sub4(a, b, c, d, out):
                # out = (a - b) - (c - d)
                tt(tt(a, b, OP.subtract), tt(c, d, OP.subtract),
                   OP.subtract, out=out)

            qv = ctile([128, G, 3])
            sub4(tt(xp, wg, OP.mult), tt(wp, xg, OP.mult),
                 tt(yp, zg, OP.mult), tt(zp, yg, OP.mult), qv[:, :, 0])
            sub4(tt(yp, wg, OP.mult), tt(wp, yg, OP.mult),
                 tt(zp, xg, OP.mult), tt(xp, zg, OP.mult), qv[:, :, 1])
            sub4(tt(zp, wg, OP.mult), tt(wp, zg, OP.mult),
                 tt(xp, yg, OP.mult), tt(yp, xg, OP.mult), qv[:, :, 2])
            qx, qy, qz = qv[:, :, 0], qv[:, :, 1], qv[:, :, 2]

            # ---- Householder basis of plane perp to qv, norm |qv| ----
            # v = qv + sign(qz)*|qv|*zhat; E1/E2 = (+/-)|qv|*(I-2vv^T/|v|^2)e_xy
            sz = sign_dve(qz)                   # off-path-ish
            aqz = tt(qz, sz, OP.mult)           # |qz|
            qq = tt(qv[:, :, :], qv[:, :, :], OP.mult, shape=[128, G, 3])
            q2 = tt(tt(qq[:, :, 0], qq[:, :, 1], OP.add), qq[:, :, 2], OP.add)
            nq = act(q2, AF.Sqrt)               # |qv|
            snq = tt(sz, nq, OP.mult)
            vz = tt(qz, snq, OP.add)
            hv2 = tt(q2, tt(nq, aqz, OP.mult), OP.add)   # |v|^2/2
            k = tt(nq, recip(hv2), OP.mult)
            vxk = tt(qx, k, OP.mult)
            vyk = tt(qy, k, OP.mult)
            E1 = ctile([128, G, 3])
            E2 = ctile([128, G, 3])
            tt(tt(qx, vxk, OP.mult), nq, OP.subtract, out=E1[:, :, 0])
            tt(qy, vxk, OP.mult, out=E1[:, :, 1])
            tt(vz, vxk, OP.mult, out=E1[:, :, 2])
            tt(qx, vyk, OP.mult, out=E2[:, :, 0])
            tt(tt(qy, vyk, OP.mult), nq, OP.subtract, out=E2[:, :, 1])
            tt(vz, vyk, OP.mult, out=E2[:, :, 2])

            # ---- identity matrix (bf16) for diag stationaries ----
            ones = cp.tile([128, 128], BF16, name="ones", tag="ones")
            nc.vector.memset(ones[:, :], 1.0)
            ident = cp.tile([128, 128], BF16, name="ident", tag="ident")
            nc.gpsimd.affine_select(out=ident[:], in_=ones[:],
                                    pattern=[[-1, 128]],
                                    compare_op=OP.is_equal, fill=0.0,
                                    base=0, channel_multiplier=1)

            acc = cp.tile([128, G], F32, name="acc", tag="acc")

            # ---- main loop over b-groups ----
            for g in range(G):
                T = dp.tile([128, ROW], BF16, name="T", tag="T")
                nc.gpsimd.dma_start(out=T[:, :],
                                    in_=point[g:BSH:G, :])

                diags = []
                for nm, E in (("d1", E1), ("d2", E2)):
                    row = []
                    for c in range(3):
                        d = gp.tile([128, 128], BF16, name=f"{nm}{c}",
                                    tag=f"{nm}{c}")
                        nc.vector.tensor_scalar(out=d[:], in0=ident[:],
                                                scalar1=E[:, g:g + 1, c],
                                                scalar2=None, op0=OP.mult)
                        row.append(d)
                    diags.append(row)

                # one 4-bank PSUM tile: [v1 | v2], each [128, N]
                pv = pp.tile([128, 2 * N], F32, name="pv", tag="pv")
                for h in range(2):
                    o = h * HALF
                    views = [T[:, o + c:o + HALF:3] for c in range(3)]
                    for j, drow in enumerate(diags):
                        col = slice(j * N + h * (N // 2),
                                    j * N + (h + 1) * (N // 2))
                        nc.tensor.matmul(out=pv[:, col], lhsT=drow[0][:],
                                         rhs=views[0], start=True, stop=False)
                        nc.tensor.matmul(out=pv[:, col], lhsT=drow[1][:],
                                         rhs=views[1], start=False, stop=False)
                        nc.tensor.matmul(out=pv[:, col], lhsT=drow[2][:],
                                         rhs=views[2], start=False, stop=True)

                sq = qp.tile([128, 2 * N], BF16, name="sq", tag="sq")
                nc.scalar.activation(out=sq[:], in_=pv[:], func=AF.Square)

                stot = qp.tile([128, N], BF16, name="stot", tag="stot")
                nc.vector.tensor_tensor(out=stot[:], in0=sq[:, 0:N],
                                        in1=sq[:, N:2 * N], op=OP.add)

                dists = qp.tile([128, N], BF16, name="dists", tag="dists")
                nc.scalar.activation(out=dists[:], in_=stot[:], func=AF.Sqrt,
                                     scale=4.0, accum_out=acc[:, g:g + 1])

            nc.sync.dma_start(out=out[:, :], in_=acc[:, :])

    nc.finalize()
    return nc


def _get_nc():
    if "nc" not in _CACHE:
        _CACHE["nc"] = build_bass()
    return _CACHE["nc"]


def kernel(pred, mode, gt, point, **run_kwargs):
    nc = _get_nc()
    in_maps = []
    for c in range(NCORES):
        sl = slice(c * BSH, (c + 1) * BSH)
        in_maps.append({
            "pred": np.ascontiguousarray(pred[sl], dtype=np.float32),
            "mode": np.ascontiguousarray(mode[sl], dtype=np.float32).reshape(BSH, 1),
            "gt": np.ascontiguousarray(gt[sl], dtype=np.float32),
            "point": np.ascontiguousarray(point[sl], dtype=np.float32).reshape(BSH, ROW),
        })
    res = run_bass_kernel_spmd(nc, in_maps, core_ids=list(range(NCORES)),
                               **run_kwargs)
    total = sum(float(r["out"].astype(np.float64).sum()) for r in res.results)
    result = np.float32(total / (B * N))
    if run_kwargs:
        return result, res
    return result


# revision 15
# speedup vs baseline: 1.2332x; 1.0954x over previous
"""Distributed Trainium2 kernel for the ADD rotation loss.

Math: the reference computes mean_{b,n} || point[b,n] @ (R_pred[b] - R_gt[b]) ||
with R_pred/R_gt rotation matrices. Because both are rotations,

    || p @ (Rp - Rg) || = 2 * | p x qv |,

where qv is the vector part of the relative quaternion q_pred * conj(q_gt).
The pred-side euler angles enter only through cos/sin, which reduce to pure
arithmetic (no arcsin/arctan2 needed); only the gt side needs real sin().
With {E1, E2} an orthogonal basis of the plane perpendicular to qv scaled to
length |qv| (from the Householder reflection that maps qv to the z axis),

    | p x qv |^2 = (p . E1)^2 + (p . E2)^2,

i.e. TWO per-point projections + squares instead of three cross components.

Per core (data-parallel over batch): cast-DMA the point shard f32->bf16 once
from HBM, project onto E1/E2 via TensorEngine matmuls whose stationary is a
*diagonal* bf16 matrix (per-batch-row coefficient on the diagonal), square on
ACT (one [128,2048] op per group), add halves on DVE, sqrt + per-row
accumulate on ACT, and emit per-row partial sums. The final tiny reduction
(8 cores x 128 x 8 values) happens on the host.
"""

import sys

for _p in ("/opt/trn_rl_repo", "/root/.axon_site/_ro/trn_rl_repo"):
    if _p not in sys.path:
        sys.path.append(_p)

import math

import numpy as np

import concourse.bacc as bacc
import concourse.tile as tile
from concourse import mybir
from concourse.bass_utils import run_bass_kernel_spmd

NCORES = 8
B = 8192
N = 1024
BSH = B // NCORES          # batch rows per core
G = BSH // 128             # b-groups of 128 rows per core
ROW = 3 * N                # elements per point row
HALF = 3 * (N // 2)        # elements per half row
F32 = mybir.dt.float32
BF16 = mybir.dt.bfloat16
OP = mybir.AluOpType
AF = mybir.ActivationFunctionType

_CACHE = {}


def build_bass():
    nc = bacc.Bacc("TRN2", target_bir_lowering=False, debug=False,
                   num_devices=NCORES)
    pred = nc.declare_dram_parameter("pred", [BSH, 4], F32, isOutput=False)
    mode = nc.declare_dram_parameter("mode", [BSH, 1], F32, isOutput=False)
    gt = nc.declare_dram_parameter("gt", [BSH, 3], F32, isOutput=False)
    point = nc.declare_dram_parameter("point", [BSH, ROW], F32, isOutput=False)
    out = nc.declare_dram_parameter("out", [128, G // 2], F32, isOutput=True)

    with tile.TileContext(nc) as tc:
        with (
            tc.tile_pool(name="coef", bufs=1) as cp,
            tc.tile_pool(name="data", bufs=3) as dp,
            tc.tile_pool(name="diag", bufs=2) as gp,
            tc.tile_pool(name="sq", bufs=2) as qp,
            tc.tile_pool(name="psum", bufs=2, space="PSUM") as pp,
        ):
            uid = [0]

            def ctile(shape, dtype=F32):
                uid[0] += 1
                return cp.tile(shape, dtype, name=f"c{uid[0]}",
                               tag=f"c{uid[0]}")

            def tt(in0, in1, op, shape=None, out=None):
                """out = in0 op in1 (DVE); returns the written AP."""
                if out is None:
                    out = ctile(shape if shape is not None else [128, G])
                nc.vector.tensor_tensor(out=out, in0=in0, in1=in1, op=op)
                return out

            def ts(in0, s1, s2, op0, op1=None, out=None, shape=None):
                if out is None:
                    out = ctile(shape if shape is not None else [128, G])
                if op1 is None:
                    nc.vector.tensor_scalar(out=out, in0=in0, scalar1=s1,
                                            scalar2=None, op0=op0)
                else:
                    nc.vector.tensor_scalar(out=out, in0=in0, scalar1=s1,
                                            scalar2=s2, op0=op0, op1=op1)
                return out

            def sign_dve(in_, shape=None):
                # sign(x) in {-1, +1} without touching ACT:
                # ((x is_ge 0) * 2) - 1
                h = ts(in_, 0.0, 2.0, OP.is_ge, OP.mult, shape=shape)
                return ts(h, -1.0, None, OP.add, out=None, shape=shape)

            def recip(in_, shape=None):
                o = ctile(shape if shape is not None else [128, G])
                nc.vector.reciprocal(out=o, in_=in_)
                return o

            _consts = {}

            def constcol(val):
                if val not in _consts:
                    uid[0] += 1
                    t = cp.tile([128, 1], F32, name=f"k{uid[0]}",
                                tag=f"k{uid[0]}")
                    nc.vector.memset(t[:, :], val)
                    _consts[val] = t
                return _consts[val]

            def act(in_, func, scale=1.0, bias=0.0, out=None, shape=None):
                if out is None:
                    out = ctile(shape if shape is not None else [128, G])
                if isinstance(bias, float) and bias != 0.0:
                    bias = constcol(bias)[:, :]
                nc.scalar.activation(out=out, in_=in_, func=func,
                                     scale=scale, bias=bias)
                return out

            # ---- coefficient inputs ----
            # Row assignment: batch row b = G*p + g lives at [partition p,
            # group g], so each load is one small fully-contiguous DMA.
            cgt = ctile([128, G, 3])
            nc.gpsimd.dma_start(out=cgt[:, :, :],
                                in_=gt[:, :].rearrange("(p g) c -> p g c", p=128))
            cpred = ctile([128, G, 4])
            nc.gpsimd.dma_start(out=cpred[:, :, :],
                                in_=pred[:, :].rearrange("(p g) c -> p g c", p=128))
            cmode = ctile([128, G, 1])
            nc.gpsimd.dma_start(out=cmode[:, :, :],
                                in_=mode[:, :].rearrange("(p g) c -> p g c", p=128))

            # gt half-angle cos/sin first: ACT loads the trig table once,
            # then switches to the sqrt set for the rest of the kernel.
            chg = act(cgt[:, :, :], AF.Sin, scale=0.5, bias=math.pi / 2,
                      shape=[128, G, 3])
            shg = act(cgt[:, :, :], AF.Sin, scale=0.5, shape=[128, G, 3])

            # ---- pred side: cos/sin of euler angles, arithmetic only ----
            m1, m2 = cpred[:, :, 0], cpred[:, :, 1]
            m3, m4 = cpred[:, :, 2], cpred[:, :, 3]
            # off-critical-path signs (inputs only)
            sgn = act(cmode[:, :, 0], AF.Sign, bias=-0.5)   # mode>0.5 -> +1
            sm3 = sign_dve(m3)
            ssm = tt(sgn, sm3, OP.mult)        # sgn*sign(m3)
            am3 = tt(m3, sm3, OP.mult)         # |m3|
            ams = tt(am3, sgn, OP.mult)        # sgn*|m3|

            msq = tt(cpred[:, :, :], cpred[:, :, :], OP.mult, shape=[128, G, 4])
            m1sq, m2sq = msq[:, :, 0], msq[:, :, 1]
            m3sq, m4sq = msq[:, :, 2], msq[:, :, 3]
            rxy = tt(m1sq, m2sq, OP.add)
            r = tt(rxy, m3sq, OP.add)

            cc = ctile([128, G, 3])   # cos(e1..e3)
            ss = ctile([128, G, 3])   # sin(e1..e3)

            # e2: sin = sgn*|m3|*rsqrt(r), cos = sqrt(rxy)*rsqrt(r)
            rsr = act(recip(r), AF.Sqrt)       # rsqrt(r)
            rt_xy = act(rxy, AF.Sqrt)          # sqrt(rxy)
            tt(rt_xy, rsr, OP.mult, out=cc[:, :, 1])
            tt(ams, rsr, OP.mult, out=ss[:, :, 1])

            # e3 (w = m3/(sin(e2)+1e-9) ~= sgn*sign(m3)*sqrt(r)):
            # cos/sin = (w, m4)/hyp(w, m4), w^2 = r
            rt = act(r, AF.Sqrt)               # sqrt(r)
            h3sq = tt(r, m4sq, OP.add)
            h3i = act(recip(h3sq), AF.Sqrt)    # rsqrt(r + m4^2)
            wh = tt(rt, h3i, OP.mult)          # sqrt(r)*h3i
            tt(wh, ssm, OP.mult, out=cc[:, :, 2])
            tt(m4, h3i, OP.mult, out=ss[:, :, 2])

            # e1: cos/sin = sign(cos2*cos3) * (m1, m2)/hyp(m1, m2)
            hyi = act(recip(rxy), AF.Sqrt)     # rsqrt(m1^2+m2^2)
            c1a = tt(m1, hyi, OP.mult)
            s1a = tt(m2, hyi, OP.mult)
            tmp = tt(cc[:, :, 1], cc[:, :, 2], OP.mult)
            sgnt = sign_dve(tmp)
            tt(c1a, sgnt, OP.mult, out=cc[:, :, 0])
            tt(s1a, sgnt, OP.mult, out=ss[:, :, 0])

            # clamp cosines into [-1, 1] so the half-angle sqrts stay real
            ts(cc, 1.0, -1.0, OP.min, OP.max, out=cc, shape=None)

            # pred half-angle: ch = sqrt((1+c)/2), sh = sign(s)*sqrt((1-c)/2)
            chp = act(cc, AF.Sqrt, scale=0.5, bias=0.5, shape=[128, G, 3])
            shab = act(cc, AF.Sqrt, scale=-0.5, bias=0.5, shape=[128, G, 3])
            ssg1 = ts(ss, 0.0, 2.0, OP.is_ge, OP.mult, shape=[128, G, 3])
            ssgn = ts(ssg1, -1.0, None, OP.add, shape=[128, G, 3])
            shp = tt(shab, ssgn, OP.mult, shape=[128, G, 3])

            # ---- quaternions: q = qx(e1) * qy(e2) * qz(e3) ----
            def quat_xyz(ch, sh):
                c1h, s1h = ch[:, :, 0], sh[:, :, 0]
                c2h, s2h = ch[:, :, 1], sh[:, :, 1]
                c3h, s3h = ch[:, :, 2], sh[:, :, 2]
                w12 = tt(c1h, c2h, OP.mult)
                x12 = tt(s1h, c2h, OP.mult)
                y12 = tt(c1h, s2h, OP.mult)
                z12 = tt(s1h, s2h, OP.mult)
                wq = tt(tt(w12, c3h, OP.mult), tt(z12, s3h, OP.mult), OP.subtract)
                xq = tt(tt(x12, c3h, OP.mult), tt(y12, s3h, OP.mult), OP.add)
                yq = tt(tt(y12, c3h, OP.mult), tt(x12, s3h, OP.mult), OP.subtract)
                zq = tt(tt(w12, s3h, OP.mult), tt(z12, c3h, OP.mult), OP.add)
                return wq, xq, yq, zq

            wp, xp, yp, zp = quat_xyz(chp, shp)
            wg, xg, yg, zg = quat_xyz(chg, shg)

            # qv = vec(q_pred * conj(q_gt))
            def sub4(a, b, c, d, out):
                # out = (a - b) - (c - d)
                tt(tt(a, b, OP.subtract), tt(c, d, OP.subtract),
                   OP.subtract, out=out)

            qv = ctile([128, G, 3])
            sub4(tt(xp, wg, OP.mult), tt(wp, xg, OP.mult),
                 tt(yp, zg, OP.mult), tt(zp, yg, OP.mult), qv[:, :, 0])
            sub4(tt(yp, wg, OP.mult), tt(wp, yg, OP.mult),
                 tt(zp, xg, OP.mult), tt(xp, zg, OP.mult), qv[:, :, 1])
            sub4(tt(zp, wg, OP.mult), tt(wp, zg, OP.mult),
                 tt(xp, yg, OP.mult), tt(yp, xg, OP.mult), qv[:, :, 2])
            qx, qy, qz = qv[:, :, 0], qv[:, :, 1], qv[:, :, 2]

            # ---- Householder basis of plane perp to qv, norm |qv| ----
            # v = qv + sign(qz)*|qv|*zhat; E1/E2 = (+/-)|qv|*(I-2vv^T/|v|^2)e_xy
            sz = sign_dve(qz)                   # off-path-ish
            aqz = tt(qz, sz, OP.mult)           # |qz|
            qq = tt(qv[:, :, :], qv[:, :, :], OP.mult, shape=[128, G, 3])
            q2 = tt(tt(qq[:, :, 0], qq[:, :, 1], OP.add), qq[:, :, 2], OP.add)
            nq = act(q2, AF.Sqrt)               # |qv|
            snq = tt(sz, nq, OP.mult)
            vz = tt(qz, snq, OP.add)
            hv2 = tt(q2, tt(nq, aqz, OP.mult), OP.add)   # |v|^2/2
            k = tt(nq, recip(hv2), OP.mult)
            vxk = tt(qx, k, OP.mult)
            vyk = tt(qy, k, OP.mult)
            E1 = ctile([128, G, 3])
            E2 = ctile([128, G, 3])
            tt(tt(qx, vxk, OP.mult), nq, OP.subtract, out=E1[:, :, 0])
            tt(qy, vxk, OP.mult, out=E1[:, :, 1])
            tt(vz, vxk, OP.mult, out=E1[:, :, 2])
            tt(qx, vyk, OP.mult, out=E2[:, :, 0])
            tt(tt(qy, vyk, OP.mult), nq, OP.subtract, out=E2[:, :, 1])
            tt(vz, vyk, OP.mult, out=E2[:, :, 2])

            # ---- identity matrix (bf16) for diag stationaries ----
            ones = cp.tile([128, 128], BF16, name="ones", tag="ones")
            nc.vector.memset(ones[:, :], 1.0)
            ident = cp.tile([128, 128], BF16, name="ident", tag="ident")
            nc.gpsimd.affine_select(out=ident[:], in_=ones[:],
                                    pattern=[[-1, 128]],
                                    compare_op=OP.is_equal, fill=0.0,
                                    base=0, channel_multiplier=1)

            acc = cp.tile([128, G // 2], F32, name="acc", tag="acc")

            # ---- main loop over b-groups ----
            for g in range(G):
                T = dp.tile([128, ROW], BF16, name="T", tag="T")
                nc.gpsimd.dma_start(out=T[:, :],
                                    in_=point[g:BSH:G, :])

                diags = []
                for nm, E in (("d1", E1), ("d2", E2)):
                    row = []
                    for c in range(3):
                        d = gp.tile([128, 128], BF16, name=f"{nm}{c}",
                                    tag=f"{nm}{c}")
                        nc.vector.tensor_scalar(out=d[:], in0=ident[:],
                                                scalar1=E[:, g:g + 1, c],
                                                scalar2=None, op0=OP.mult)
                        row.append(d)
                    diags.append(row)

                # one 4-bank PSUM tile: [v1 | v2], each [128, N]
                pv = pp.tile([128, 2 * N], F32, name="pv", tag="pv")
                for h in range(2):
                    o = h * HALF
                    views = [T[:, o + c:o + HALF:3] for c in range(3)]
                    for j, drow in enumerate(diags):
                        col = slice(j * N + h * (N // 2),
                                    j * N + (h + 1) * (N // 2))
                        nc.tensor.matmul(out=pv[:, col], lhsT=drow[0][:],
                                         rhs=views[0], start=True, stop=False)
                        nc.tensor.matmul(out=pv[:, col], lhsT=drow[1][:],
                                         rhs=views[1], start=False, stop=False)
                        nc.tensor.matmul(out=pv[:, col], lhsT=drow[2][:],
                                         rhs=views[2], start=False, stop=True)

                sq = qp.tile([128, 2 * N], BF16, name="sq", tag="sq")
                nc.scalar.activation(out=sq[:], in_=pv[:], func=AF.Square)

                if g % 2 == 0:
                    stot2 = qp.tile([128, 2 * N], BF16, name="stot2",
                                    tag="stot2")
                nc.vector.tensor_tensor(out=stot2[:, (g % 2) * N:(g % 2 + 1) * N],
                                        in0=sq[:, 0:N],
                                        in1=sq[:, N:2 * N], op=OP.add)

                if g % 2 == 1:
                    dists = qp.tile([128, 2 * N], BF16, name="dists",
                                    tag="dists")
                    nc.scalar.activation(out=dists[:], in_=stot2[:],
                                         func=AF.Sqrt, scale=4.0,
                                         accum_out=acc[:, g // 2:g // 2 + 1])

            nc.sync.dma_start(out=out[:, :], in_=acc[:, :])

    nc.finalize()
    return nc


def _get_nc():
    if "nc" not in _CACHE:
        _CACHE["nc"] = build_bass()
    return _CACHE["nc"]


def kernel(pred, mode, gt, point, **run_kwargs):
    nc = _get_nc()
    in_maps = []
    for c in range(NCORES):
        sl = slice(c * BSH, (c + 1) * BSH)
        in_maps.append({
            "pred": np.ascontiguousarray(pred[sl], dtype=np.float32),
            "mode": np.ascontiguousarray(mode[sl], dtype=np.float32).reshape(BSH, 1),
            "gt": np.ascontiguousarray(gt[sl], dtype=np.float32),
            "point": np.ascontiguousarray(point[sl], dtype=np.float32).reshape(BSH, ROW),
        })
    res = run_bass_kernel_spmd(nc, in_maps, core_ids=list(range(NCORES)),
                               **run_kwargs)
    total = sum(float(r["out"].astype(np.float64).sum()) for r in res.results)
    result = np.float32(total / (B * N))
    if run_kwargs:
        return result, res
    return result


# revision 16
# speedup vs baseline: 1.3328x; 1.0808x over previous
"""Distributed Trainium2 kernel for the ADD rotation loss.

Math: the reference computes mean_{b,n} || point[b,n] @ (R_pred[b] - R_gt[b]) ||
with R_pred/R_gt rotation matrices. Because both are rotations,

    || p @ (Rp - Rg) || = 2 * | p x qv |,

where qv is the vector part of the relative quaternion q_pred * conj(q_gt).
The pred-side euler angles enter only through cos/sin, which reduce to pure
arithmetic (no arcsin/arctan2 needed); only the gt side needs real sin().
With {E1, E2} an orthogonal basis of the plane perpendicular to qv scaled to
length |qv| (from the Householder reflection that maps qv to the z axis),

    | p x qv |^2 = (p . E1)^2 + (p . E2)^2,

i.e. TWO per-point projections + squares instead of three cross components.

Per core (data-parallel over batch): cast-DMA the point shard f32->bf16 once
from HBM, project onto E1/E2 via TensorEngine matmuls whose stationary is a
*diagonal* bf16 matrix (per-batch-row coefficient on the diagonal), square on
ACT (one [128,2048] op per group), add halves on DVE, sqrt + per-row
accumulate on ACT, and emit per-row partial sums. The final tiny reduction
(8 cores x 128 x 8 values) happens on the host.
"""

import sys

for _p in ("/opt/trn_rl_repo", "/root/.axon_site/_ro/trn_rl_repo"):
    if _p not in sys.path:
        sys.path.append(_p)

import math

import numpy as np

import concourse.bacc as bacc
import concourse.tile as tile
from concourse import mybir
from concourse.bass_utils import run_bass_kernel_spmd

NCORES = 8
B = 8192
N = 1024
BSH = B // NCORES          # batch rows per core
G = BSH // 128             # b-groups of 128 rows per core
ROW = 3 * N                # elements per point row
HALF = 3 * (N // 2)        # elements per half row
F32 = mybir.dt.float32
BF16 = mybir.dt.bfloat16
OP = mybir.AluOpType
AF = mybir.ActivationFunctionType

_CACHE = {}


def build_bass():
    nc = bacc.Bacc("TRN2", target_bir_lowering=False, debug=False,
                   num_devices=NCORES)
    pred = nc.declare_dram_parameter("pred", [BSH, 4], F32, isOutput=False)
    mode = nc.declare_dram_parameter("mode", [BSH, 1], F32, isOutput=False)
    gt = nc.declare_dram_parameter("gt", [BSH, 3], F32, isOutput=False)
    point = nc.declare_dram_parameter("point", [BSH, ROW], F32, isOutput=False)
    out = nc.declare_dram_parameter("out", [128, G // 2], F32, isOutput=True)

    with tile.TileContext(nc) as tc:
        with (
            tc.tile_pool(name="coef", bufs=1) as cp,
            tc.tile_pool(name="data", bufs=3) as dp,
            tc.tile_pool(name="diag", bufs=2) as gp,
            tc.tile_pool(name="sq", bufs=2) as qp,
            tc.tile_pool(name="psum", bufs=2, space="PSUM") as pp,
        ):
            uid = [0]

            def ctile(shape, dtype=F32):
                uid[0] += 1
                return cp.tile(shape, dtype, name=f"c{uid[0]}",
                               tag=f"c{uid[0]}")

            def tt(in0, in1, op, shape=None, out=None):
                """out = in0 op in1 (DVE); returns the written AP."""
                if out is None:
                    out = ctile(shape if shape is not None else [128, G])
                nc.vector.tensor_tensor(out=out, in0=in0, in1=in1, op=op)
                return out

            def ts(in0, s1, s2, op0, op1=None, out=None, shape=None):
                if out is None:
                    out = ctile(shape if shape is not None else [128, G])
                if op1 is None:
                    nc.vector.tensor_scalar(out=out, in0=in0, scalar1=s1,
                                            scalar2=None, op0=op0)
                else:
                    nc.vector.tensor_scalar(out=out, in0=in0, scalar1=s1,
                                            scalar2=s2, op0=op0, op1=op1)
                return out

            def sign_dve(in_, shape=None):
                # sign(x) in {-1, +1} without touching ACT:
                # ((x is_ge 0) * 2) - 1
                h = ts(in_, 0.0, 2.0, OP.is_ge, OP.mult, shape=shape)
                return ts(h, -1.0, None, OP.add, out=None, shape=shape)

            def recip(in_, shape=None):
                o = ctile(shape if shape is not None else [128, G])
                nc.vector.reciprocal(out=o, in_=in_)
                return o

            _consts = {}

            def constcol(val):
                if val not in _consts:
                    uid[0] += 1
                    t = cp.tile([128, 1], F32, name=f"k{uid[0]}",
                                tag=f"k{uid[0]}")
                    nc.vector.memset(t[:, :], val)
                    _consts[val] = t
                return _consts[val]

            def act(in_, func, scale=1.0, bias=0.0, out=None, shape=None):
                if out is None:
                    out = ctile(shape if shape is not None else [128, G])
                if isinstance(bias, float) and bias != 0.0:
                    bias = constcol(bias)[:, :]
                nc.scalar.activation(out=out, in_=in_, func=func,
                                     scale=scale, bias=bias)
                return out

            # ---- coefficient inputs ----
            # Row assignment: batch row b = G*p + g lives at [partition p,
            # group g], so each load is one small fully-contiguous DMA.
            cgt = ctile([128, G, 3])
            nc.gpsimd.dma_start(out=cgt[:, :, :],
                                in_=gt[:, :].rearrange("(p g) c -> p g c", p=128))
            cpred = ctile([128, G, 4])
            nc.gpsimd.dma_start(out=cpred[:, :, :],
                                in_=pred[:, :].rearrange("(p g) c -> p g c", p=128))
            cmode = ctile([128, G, 1])
            nc.gpsimd.dma_start(out=cmode[:, :, :],
                                in_=mode[:, :].rearrange("(p g) c -> p g c", p=128))

            # gt half-angle cos/sin first: ACT loads the trig table once,
            # then switches to the sqrt set for the rest of the kernel.
            chg = act(cgt[:, :, :], AF.Sin, scale=0.5, bias=math.pi / 2,
                      shape=[128, G, 3])
            shg = act(cgt[:, :, :], AF.Sin, scale=0.5, shape=[128, G, 3])

            # ---- pred side: cos/sin of euler angles, arithmetic only ----
            m1, m2 = cpred[:, :, 0], cpred[:, :, 1]
            m3, m4 = cpred[:, :, 2], cpred[:, :, 3]
            # off-critical-path signs (inputs only)
            sgn = act(cmode[:, :, 0], AF.Sign, bias=-0.5)   # mode>0.5 -> +1
            sm3 = sign_dve(m3)
            ssm = tt(sgn, sm3, OP.mult)        # sgn*sign(m3)
            am3 = tt(m3, sm3, OP.mult)         # |m3|
            ams = tt(am3, sgn, OP.mult)        # sgn*|m3|

            msq = tt(cpred[:, :, :], cpred[:, :, :], OP.mult, shape=[128, G, 4])
            m1sq, m2sq = msq[:, :, 0], msq[:, :, 1]
            m3sq, m4sq = msq[:, :, 2], msq[:, :, 3]
            rxy = tt(m1sq, m2sq, OP.add)
            r = tt(rxy, m3sq, OP.add)

            cc = ctile([128, G, 3])   # cos(e1..e3)
            ss = ctile([128, G, 3])   # sin(e1..e3)

            # e2: sin = sgn*|m3|*rsqrt(r), cos = sqrt(rxy)*rsqrt(r)
            rsr = act(recip(r), AF.Sqrt)       # rsqrt(r)
            rt_xy = act(rxy, AF.Sqrt)          # sqrt(rxy)
            tt(rt_xy, rsr, OP.mult, out=cc[:, :, 1])
            tt(ams, rsr, OP.mult, out=ss[:, :, 1])

            # e3 (w = m3/(sin(e2)+1e-9) ~= sgn*sign(m3)*sqrt(r)):
            # cos/sin = (w, m4)/hyp(w, m4), w^2 = r
            rt = act(r, AF.Sqrt)               # sqrt(r)
            h3sq = tt(r, m4sq, OP.add)
            h3i = act(recip(h3sq), AF.Sqrt)    # rsqrt(r + m4^2)
            wh = tt(rt, h3i, OP.mult)          # sqrt(r)*h3i
            tt(wh, ssm, OP.mult, out=cc[:, :, 2])
            tt(m4, h3i, OP.mult, out=ss[:, :, 2])

            # e1: cos/sin = sign(cos2*cos3) * (m1, m2)/hyp(m1, m2)
            hyi = act(recip(rxy), AF.Sqrt)     # rsqrt(m1^2+m2^2)
            c1a = tt(m1, hyi, OP.mult)
            s1a = tt(m2, hyi, OP.mult)
            tmp = tt(cc[:, :, 1], cc[:, :, 2], OP.mult)
            sgnt = sign_dve(tmp)
            tt(c1a, sgnt, OP.mult, out=cc[:, :, 0])
            tt(s1a, sgnt, OP.mult, out=ss[:, :, 0])

            # clamp cosines into [-1, 1] so the half-angle sqrts stay real
            ts(cc, 1.0, -1.0, OP.min, OP.max, out=cc, shape=None)

            # pred half-angle: ch = sqrt((1+c)/2), sh = sign(s)*sqrt((1-c)/2)
            chp = act(cc, AF.Sqrt, scale=0.5, bias=0.5, shape=[128, G, 3])
            shab = act(cc, AF.Sqrt, scale=-0.5, bias=0.5, shape=[128, G, 3])
            ssg1 = ts(ss, 0.0, 2.0, OP.is_ge, OP.mult, shape=[128, G, 3])
            ssgn = ts(ssg1, -1.0, None, OP.add, shape=[128, G, 3])
            shp = tt(shab, ssgn, OP.mult, shape=[128, G, 3])

            # ---- quaternions: q = qx(e1) * qy(e2) * qz(e3) ----
            def quat_xyz(ch, sh):
                c1h, s1h = ch[:, :, 0], sh[:, :, 0]
                c2h, s2h = ch[:, :, 1], sh[:, :, 1]
                c3h, s3h = ch[:, :, 2], sh[:, :, 2]
                w12 = tt(c1h, c2h, OP.mult)
                x12 = tt(s1h, c2h, OP.mult)
                y12 = tt(c1h, s2h, OP.mult)
                z12 = tt(s1h, s2h, OP.mult)
                wq = tt(tt(w12, c3h, OP.mult), tt(z12, s3h, OP.mult), OP.subtract)
                xq = tt(tt(x12, c3h, OP.mult), tt(y12, s3h, OP.mult), OP.add)
                yq = tt(tt(y12, c3h, OP.mult), tt(x12, s3h, OP.mult), OP.subtract)
                zq = tt(tt(w12, s3h, OP.mult), tt(z12, c3h, OP.mult), OP.add)
                return wq, xq, yq, zq

            wp, xp, yp, zp = quat_xyz(chp, shp)
            wg, xg, yg, zg = quat_xyz(chg, shg)

            # qv = vec(q_pred * conj(q_gt))
            def sub4(a, b, c, d, out):
                # out = (a - b) - (c - d)
                tt(tt(a, b, OP.subtract), tt(c, d, OP.subtract),
                   OP.subtract, out=out)

            qv = ctile([128, G, 3])
            sub4(tt(xp, wg, OP.mult), tt(wp, xg, OP.mult),
                 tt(yp, zg, OP.mult), tt(zp, yg, OP.mult), qv[:, :, 0])
            sub4(tt(yp, wg, OP.mult), tt(wp, yg, OP.mult),
                 tt(zp, xg, OP.mult), tt(xp, zg, OP.mult), qv[:, :, 1])
            sub4(tt(zp, wg, OP.mult), tt(wp, zg, OP.mult),
                 tt(xp, yg, OP.mult), tt(yp, xg, OP.mult), qv[:, :, 2])
            qx, qy, qz = qv[:, :, 0], qv[:, :, 1], qv[:, :, 2]

            # ---- Householder basis of plane perp to qv, norm |qv| ----
            # v = qv + sign(qz)*|qv|*zhat; E1/E2 = (+/-)|qv|*(I-2vv^T/|v|^2)e_xy
            sz = sign_dve(qz)                   # off-path-ish
            aqz = tt(qz, sz, OP.mult)           # |qz|
            qq = tt(qv[:, :, :], qv[:, :, :], OP.mult, shape=[128, G, 3])
            q2 = tt(tt(qq[:, :, 0], qq[:, :, 1], OP.add), qq[:, :, 2], OP.add)
            nq = act(q2, AF.Sqrt)               # |qv|
            snq = tt(sz, nq, OP.mult)
            vz = tt(qz, snq, OP.add)
            hv2 = tt(q2, tt(nq, aqz, OP.mult), OP.add)   # |v|^2/2
            k = tt(nq, recip(hv2), OP.mult)
            vxk = tt(qx, k, OP.mult)
            vyk = tt(qy, k, OP.mult)
            E1 = ctile([128, G, 3])
            E2 = ctile([128, G, 3])
            tt(tt(qx, vxk, OP.mult), nq, OP.subtract, out=E1[:, :, 0])
            tt(qy, vxk, OP.mult, out=E1[:, :, 1])
            tt(vz, vxk, OP.mult, out=E1[:, :, 2])
            tt(qx, vyk, OP.mult, out=E2[:, :, 0])
            tt(tt(qy, vyk, OP.mult), nq, OP.subtract, out=E2[:, :, 1])
            tt(vz, vyk, OP.mult, out=E2[:, :, 2])

            # ---- identity matrix (bf16) for diag stationaries ----
            ones = cp.tile([128, 128], BF16, name="ones", tag="ones")
            nc.vector.memset(ones[:, :], 1.0)
            ident = cp.tile([128, 128], BF16, name="ident", tag="ident")
            nc.gpsimd.affine_select(out=ident[:], in_=ones[:],
                                    pattern=[[-1, 128]],
                                    compare_op=OP.is_equal, fill=0.0,
                                    base=0, channel_multiplier=1)

            acc = cp.tile([128, G // 2], F32, name="acc", tag="acc")

            # ---- main loop over b-groups ----
            for g in range(G):
                T = dp.tile([128, ROW], BF16, name="T", tag="T")
                nc.gpsimd.dma_start(out=T[:, :],
                                    in_=point[g:BSH:G, :])

                diags = []
                for nm, E in (("d1", E1), ("d2", E2)):
                    row = []
                    for c in range(3):
                        d = gp.tile([128, 128], BF16, name=f"{nm}{c}",
                                    tag=f"{nm}{c}")
                        nc.vector.tensor_scalar(out=d[:], in0=ident[:],
                                                scalar1=E[:, g:g + 1, c],
                                                scalar2=None, op0=OP.mult)
                        row.append(d)
                    diags.append(row)

                # one 4-bank PSUM tile: [v1 | v2], each [128, N]
                pv = pp.tile([128, 2 * N], F32, name="pv", tag="pv")
                for h in range(2):
                    # host pre-deinterleaves each row to [3, N]: plane c at
                    # [c*N : (c+1)*N], so every view is unit-stride
                    views = [T[:, c * N + h * (N // 2):
                                c * N + (h + 1) * (N // 2)] for c in range(3)]
                    for j, drow in enumerate(diags):
                        col = slice(j * N + h * (N // 2),
                                    j * N + (h + 1) * (N // 2))
                        nc.tensor.matmul(out=pv[:, col], lhsT=drow[0][:],
                                         rhs=views[0], start=True, stop=False)
                        nc.tensor.matmul(out=pv[:, col], lhsT=drow[1][:],
                                         rhs=views[1], start=False, stop=False)
                        nc.tensor.matmul(out=pv[:, col], lhsT=drow[2][:],
                                         rhs=views[2], start=False, stop=True)

                sq = qp.tile([128, 2 * N], BF16, name="sq", tag="sq")
                nc.scalar.activation(out=sq[:], in_=pv[:], func=AF.Square)

                if g % 2 == 0:
                    stot2 = qp.tile([128, 2 * N], BF16, name="stot2",
                                    tag="stot2")
                nc.vector.tensor_tensor(out=stot2[:, (g % 2) * N:(g % 2 + 1) * N],
                                        in0=sq[:, 0:N],
                                        in1=sq[:, N:2 * N], op=OP.add)

                if g % 2 == 1:
                    dists = qp.tile([128, 2 * N], BF16, name="dists",
                                    tag="dists")
                    nc.scalar.activation(out=dists[:], in_=stot2[:],
                                         func=AF.Sqrt, scale=4.0,
                                         accum_out=acc[:, g // 2:g // 2 + 1])

            nc.sync.dma_start(out=out[:, :], in_=acc[:, :])

    nc.finalize()
    return nc


def _get_nc():
    if "nc" not in _CACHE:
        _CACHE["nc"] = build_bass()
    return _CACHE["nc"]


def kernel(pred, mode, gt, point, **run_kwargs):
    nc = _get_nc()
    in_maps = []
    for c in range(NCORES):
        sl = slice(c * BSH, (c + 1) * BSH)
        in_maps.append({
            "pred": np.ascontiguousarray(pred[sl], dtype=np.float32),
            "mode": np.ascontiguousarray(mode[sl], dtype=np.float32).reshape(BSH, 1),
            "gt": np.ascontiguousarray(gt[sl], dtype=np.float32),
            "point": np.ascontiguousarray(
                np.asarray(point[sl], dtype=np.float32)
                .reshape(BSH, N, 3).transpose(0, 2, 1)).reshape(BSH, ROW),
        })
    res = run_bass_kernel_spmd(nc, in_maps, core_ids=list(range(NCORES)),
                               **run_kwargs)
    total = sum(float(r["out"].astype(np.float64).sum()) for r in res.results)
    result = np.float32(total / (B * N))
    if run_kwargs:
        return result, res
    return result
